# revision 1
# baseline (speedup 1.0000x reference)
"""CharCNN embedding kernel for 8 Trainium2 NeuronCores (pure data parallel).

Math restructuring: CHAR_VOCAB == 128 == PE partition count, so the char
embedding + all three Conv1d branches collapse into one-hot matmuls with
fused tables Phi_j = char_emb @ w_k[:, :, j].T stacked over branches.
For output position l of a word:  conv_out[:, l] = sum_j Phi_j[char[l+j], :].

Per-core pipeline (2048 words, processed as 128 chunks of 16 words = 512
onehot columns):
  1. one indirect-DMA gather of char_table rows -> chars (128, 512) int32
     (partition p holds words [16p, 16p+16), 32 chars each), int->MM_DT
     convert on DVE
  2. chunk c's chars broadcast to all 128 partitions via a selector matmul
     (lhsT = e_c column of identity, free-broadcast to (128,128))
  3. onehot = is_equal(chars_bcast, iota partition column) on DVE -> {0,1}
     exactly.  Onehot tiles carry a 4-column halo (next chunk's first 4
     cols) so tap j's rhs is the contiguous slice oh[:, j:j+512]; garbage
     from word-boundary crossings lands only at l >= 32-j, which the
     pre-reduce masks kill.
  4. 5 accumulating matmuls (tap j) into PSUM (120, 1024) per 32-word block
  5. mask invalid tail positions (branch3: l>=30, branch5: l>=28) by adding
     a per-partition -1e30 column (DVE tensor_scalar), reduce_max over l
     (DVE) -> feats
  6. relu(feats + bias) via DVE tensor_scalar (add then max 0), final
     linear as a K=120 matmul, + lin_b via tensor_tensor add, DMA out.

max(relu(y+b)) == relu(max(y)+b) since both are monotone, so ReLU+bias run
on the (120, 32)-reduced features instead of the full conv map.

Engine discipline: walrus codegen accepts at most ONE semaphore wait per
engine instruction, so the dataflow keeps each consumer dependent on at
most one foreign engine: PE does all matmuls, DVE does everything touching
PSUM/features, and one-time "pre-touch" ops absorb the const-DMA waits so
steady-state instructions never wait on DMA lanes.
"""

import os
import sys

for _p in ("/root/.axon_site", "/root/.axon_site/_ro/trn_rl_repo",
           "/root/.axon_site/_ro/pypackages", "/opt/trn_rl_repo"):
    if os.path.isdir(_p) and _p not in sys.path:
        sys.path.append(_p)

import numpy as np

import concourse.bass as bass
import concourse.mybir as mybir
import concourse.tile as tile
from concourse.tile import add_dep_helper
from concourse.tile_scheduler import N_PROCS
from concourse.vector_clock import ScopedClock, VectorClock
from concourse.bass_utils import run_bass_kernel_spmd

dt = mybir.dt
AF = mybir.ActivationFunctionType
ALU = mybir.AluOpType

N_CORES = 8
B, S = 64, 256
W = (B * S) // N_CORES          # words per core: 2048
L = 32                          # max word length
V = 128                         # char vocab
F_TOT = 120                     # 30 + 40 + 50 filters
EMB = 50                        # output embed size
VOCAB = 50000

WPC = 16                        # words per chunk (512 onehot cols)
CW = WPC * L                    # 512
N_CHUNK = W // WPC              # 128 chunks
N_BLOCK = N_CHUNK // 2          # 64 blocks of 32 words (2-bank PSUM)

# bf16 rather than float32r: both run 1 col/cycle on the PE, but walrus
# lowers a float32r matmul to a single self-loading instruction with only
# ONE semaphore-wait slot, and psum-slot reuse forces {PE-WAW, DVE} double
# waits on the first matmul of each accumulation group. bf16 matmuls lower
# to LDWEIGHTS+MATMUL with a wait slot each.
MM_DT = dt.bfloat16

_PROGRAM_CACHE = {}
_ROLES = {}


class OneWaitTileContext(tile.TileContext):
    """TileContext whose teardown drain obeys walrus's one-semaphore-wait-
    per-instruction limit: the global drain's vector-clock waits are split
    across one nop per logical processor before a wait-free drain."""

    def _drain_and_barrier(self, tick_clock, wait_clock):
        gc = tick_clock.global_clock
        for p in range(N_PROCS):
            tick = gc.peek_next(p) - 1
            if tick <= 0:
                continue
            vec = [0] * N_PROCS
            vec[p] = tick
            nop = self.nc.sync.nop(nofuse=True)
            wait_clock.add_sem_waits(
                nop.ins, ScopedClock({None: VectorClock(vec)})
            )
        self.nc.sync.drain()
        self.nc.all_engine_barrier()
        assert self.sems is not None
        popped = self.nc._tile_sem_poison_stack.pop()
        assert popped is self._sem_poison
        self.nc.clear_and_free_semaphores(list(self.sems.allocated().values()))
        self.nc.all_engine_barrier()


def _build_program():
    if "nc" in _PROGRAM_CACHE:
        return _PROGRAM_CACHE["nc"]

    nc = bass.Bass()
    # weights packed in one tensor (one DMA -> one DMA-lane semaphore that a
    # single PE pre-touch matmul absorbs): [phi | identity | lin_w.T]
    # vector consts likewise for the DVE: [iota | bias | maska | maskb | linb]
    widx_d = nc.dram_tensor("widx", (128, WPC), dt.int32, kind="ExternalInput")
    ctab_d = nc.dram_tensor("char_table", (VOCAB, L), dt.int32, kind="ExternalInput")
    wpack_d = nc.dram_tensor("wpack", (V, 5 * V + V + EMB), MM_DT,
                             kind="ExternalInput")
    vpack_d = nc.dram_tensor("vpack", (128, 4 + EMB), dt.float32,
                             kind="ExternalInput")
    out_d = nc.dram_tensor("out", (W, EMB), dt.float32, kind="ExternalOutput")

    with OneWaitTileContext(nc) as tc:
        with (
            tc.tile_pool(name="consts", bufs=1) as consts,
            tc.tile_pool(name="ohp", bufs=4) as ohp,
            tc.tile_pool(name="psb", bufs=3, space="PSUM") as psb,
            tc.tile_pool(name="psm", bufs=2, space="PSUM") as psm,
            tc.tile_pool(name="psl", bufs=1, space="PSUM") as psl,
        ):
            wpack_sb = consts.tile((V, 5 * V + V + EMB), MM_DT)
            nc.sync.dma_start(wpack_sb[:], wpack_d[:])
            vpack_sb = consts.tile((128, 4 + EMB), dt.float32)
            nc.sync.dma_start(vpack_sb[:], vpack_d[:])
            widx_sb = consts.tile((128, WPC), dt.int32)
            nc.sync.dma_start(widx_sb[:], widx_d[:])
            phi_sb = wpack_sb[:, 0 : 5 * V]
            ident_sb = wpack_sb[:, 5 * V : 5 * V + V]
            linw_sb = wpack_sb[0:F_TOT, 5 * V + V : 5 * V + V + EMB]
            iota_sb = vpack_sb[:, 0:1]
            bias_sb = vpack_sb[0:F_TOT, 1:2]
            maska_sb = vpack_sb[0:F_TOT, 2:3]
            maskb_sb = vpack_sb[0:F_TOT, 3:4]
            linb_sb = vpack_sb[:, 4 : 4 + EMB]

            feats = consts.tile((F_TOT, W // WPC * 32), dt.float32)
            featsr = consts.tile((F_TOT, W // WPC * 32), MM_DT)
            # staged output: word g*128+p lives at [p, g*EMB:(g+1)*EMB];
            # one final DMA scatters it so no DMA lane is ever reused
            # (a reused lane adds a FIFO wait on top of the data wait,
            # and walrus allows one wait per DMA instruction)
            out_stage = consts.tile((128, (W // 128) * EMB), dt.float32)

            chars_i = consts.tile((128, CW), dt.int32)
            chars_f = consts.tile((128, CW), MM_DT)
            # one indirect DMA per word slot: hardware honors only a single
            # gathered row per partition per instruction (offset AP (128,1))
            for i in range(WPC):
                nc.gpsimd.indirect_dma_start(
                    out=chars_i[:, i * L : (i + 1) * L],
                    out_offset=None,
                    in_=ctab_d[:],
                    in_offset=bass.IndirectOffsetOnAxis(
                        ap=widx_sb[:, i : i + 1], axis=0),
                )
            for i in range(WPC):
                nc.vector.tensor_copy(chars_f[:, i * L : (i + 1) * L],
                                      chars_i[:, i * L : (i + 1) * L])

            # One-time pre-touch ops so steady-state instructions never carry
            # a DMA-lane wait on top of their data-dependency wait (walrus
            # allows only one semaphore wait per engine instruction).
            scratch = consts.tile((128, 1), dt.float32)
            nc.vector.tensor_copy(scratch[:, 0:1], vpack_sb[:, 0:1])
            pt = psb.tile((128, CW), dt.float32, tag="pb")
            nc.tensor.matmul(pt[0:1, 0:1], lhsT=wpack_sb[:, 0:1],
                             rhs=wpack_sb[:, 0:1], start=True, stop=True)

            oh_tiles = {}
            last_reduce = {}
            last_halo = None
            for blk in range(N_BLOCK + 1):
                if blk < N_BLOCK:
                    for h in range(2):
                        c = 2 * blk + h
                        pb = psb.tile((128, CW), dt.float32, tag="pb")
                        # broadcast chars row c to all partitions (e_c outer)
                        bc = nc.tensor.matmul(
                            pb[:],
                            lhsT=ident_sb[:, c : c + 1].to_broadcast((V, V)),
                            rhs=chars_f[:],
                            start=True,
                            stop=True,
                        )
                        _ROLES[bc.ins.name] = ("bcast", c)
                        # onehot(v, pos) = (chars_bcast == iota_v), exact
                        # {0,1} in bf16, straight off PSUM on the DVE.  Only
                        # PE and DVE appear in the steady loop: a third
                        # engine's cross-observations cannot be kept to
                        # walrus's one-semaphore-wait-per-instruction limit.
                        oh = ohp.tile((128, CW + 4), MM_DT, tag="oh")
                        ile = nc.vector.tensor_scalar(
                            out=oh[:, 0:CW], in0=pb[:], scalar1=iota_sb[:],
                            scalar2=None, op0=ALU.is_equal,
                        )
                        if c > 0:
                            hw_ = nc.vector.tensor_scalar(
                                out=oh_tiles[c - 1][:, CW : CW + 4],
                                in0=pb[:, 0:4], scalar1=iota_sb[:],
                                scalar2=None, op0=ALU.is_equal,
                            )
                            if h == 0:
                                # completes oh(2*blk-1) — the newest onehot
                                # this iteration's taps (block blk-1) read
                                last_halo = hw_
                        oh_tiles[c] = oh
                    if blk == N_BLOCK - 1:
                        final_memset = nc.vector.memset(
                            oh_tiles[2 * blk + 1][:, CW : CW + 4], 0.0)
                else:
                    last_halo = final_memset

                pblk = blk - 1
                if pblk < 0:
                    continue
                # Dep-carrier: a tiny standalone LDWEIGHTS that absorbs every
                # DVE tick this block's tap matmuls need (onehot halo writes
                # and the WAR on the psum slot's previous reader, the reduce
                # two blocks back). Walrus allows one semaphore wait per
                # instruction, and the first tap already carries a PSUM-drain
                # wait on the PE's own semaphore, so its DVE wait must be
                # observed earlier on the PE queue.
                ldw = nc.tensor.ldweights(weights=wpack_sb[:, 0:1])
                add_dep_helper(ldw.ins, last_halo.ins, reason="tap ACT absorb")
                if pblk - 2 in last_reduce:
                    ldw2 = nc.tensor.ldweights(weights=wpack_sb[:, 1:2])
                    add_dep_helper(ldw2.ins, last_reduce.pop(pblk - 2).ins,
                                   reason="pm-slot WAR absorb")
                    add_dep_helper(ldw.ins, ldw2.ins, reason="order carriers")
                pm = psm.tile((128, 2 * CW), dt.float32, tag="pm")
                first_tap = None
                for h in range(2):
                    oh = oh_tiles.pop(2 * pblk + h)
                    for j in range(5):
                        mm = nc.tensor.matmul(
                            pm[:, h * CW : (h + 1) * CW],
                            lhsT=phi_sb[:, j * V : (j + 1) * V],
                            rhs=oh[:, j : j + CW],
                            start=(j == 0),
                            stop=(j == 4),
                        )
                        _ROLES[mm.ins.name] = ("tap", pblk, h, j)
                        if first_tap is None:
                            first_tap = mm
                            add_dep_helper(mm.ins, ldw.ins,
                                           reason="order after carrier")
                pm3b = pm[:].rearrange("p (w l) -> p w l", l=L)
                # invalid tail positions get -1e30 before the max (garbage
                # there comes from taps whose l+j crossed a word boundary)
                nc.vector.tensor_scalar(
                    out=pm3b[0:F_TOT, :, 28:30], in0=pm3b[0:F_TOT, :, 28:30],
                    scalar1=maska_sb[:], scalar2=None, op0=ALU.add,
                )
                nc.vector.tensor_scalar(
                    out=pm3b[0:F_TOT, :, 30:32], in0=pm3b[0:F_TOT, :, 30:32],
                    scalar1=maskb_sb[:], scalar2=None, op0=ALU.add,
                )
                last_reduce[pblk] = nc.vector.tensor_reduce(
                    out=feats[0:F_TOT, pblk * 32 : (pblk + 1) * 32],
                    in_=pm3b[0:F_TOT, :, :],
                    axis=mybir.AxisListType.X,
                    op=ALU.max,
                )
                if pblk % 4 == 3:
                    g = pblk // 4
                    gs = slice(g * 128, (g + 1) * 128)
                    # featsr = relu(feats + bias), rounded to the matmul dtype
                    nc.vector.tensor_scalar(
                        out=featsr[:, gs], in0=feats[0:F_TOT, gs],
                        scalar1=bias_sb[:], scalar2=0.0,
                        op0=ALU.add, op1=ALU.max,
                    )
                    pl = psl.tile((128, EMB), dt.float32, tag="pl")
                    lm = nc.tensor.matmul(
                        pl[:],
                        lhsT=featsr[:, gs],
                        rhs=linw_sb[:],
                        start=True,
                        stop=True,
                    )
                    _ROLES[lm.ins.name] = ("linear", g)
                    nc.vector.tensor_add(
                        out_stage[:, g * EMB : (g + 1) * EMB], pl[:], linb_sb[:]
                    )

            nc.sync.dma_start(
                out_d[:].rearrange("(g p) e -> p g e", p=128),
                out_stage[:].rearrange("p (g e) -> p g e", e=EMB),
            )

    _PROGRAM_CACHE["nc"] = nc
    _PROGRAM_CACHE["roles"] = _ROLES
    return nc


def _host_prep(inputs):
    word_idxs = np.asarray(inputs["word_idxs"])
    char_table = np.ascontiguousarray(np.asarray(inputs["char_table"], dtype=np.int32))
    char_emb = np.asarray(inputs["char_emb"], dtype=np.float32)
    w1 = np.asarray(inputs["w1"], dtype=np.float32)
    w3 = np.asarray(inputs["w3"], dtype=np.float32)
    w5 = np.asarray(inputs["w5"], dtype=np.float32)
    b1 = np.asarray(inputs["b1"], dtype=np.float32)
    b3 = np.asarray(inputs["b3"], dtype=np.float32)
    b5 = np.asarray(inputs["b5"], dtype=np.float32)
    lin_w = np.asarray(inputs["lin_w"], dtype=np.float32)
    lin_b = np.asarray(inputs["lin_b"], dtype=np.float32)

    mmnp = dt.np(MM_DT)
    # phi tap blocks padded to 128 filters so bf16 FWL fast-weight-load fires
    phi = np.zeros((V, 5 * V), dtype=np.float32)
    phi[:, 0:30] = char_emb @ w1[:, :, 0].T
    for j in range(3):
        phi[:, j * V + 30 : j * V + 70] = char_emb @ w3[:, :, j].T
    for j in range(5):
        phi[:, j * V + 70 : j * V + 120] = char_emb @ w5[:, :, j].T

    wpack = np.zeros((V, 5 * V + V + EMB), dtype=np.float32)
    wpack[:, 0 : 5 * V] = phi
    wpack[:, 5 * V : 5 * V + V] = np.eye(V, dtype=np.float32)
    wpack[0:F_TOT, 5 * V + V :] = lin_w.T

    vpack = np.zeros((128, 4 + EMB), dtype=np.float32)
    vpack[:, 0] = np.arange(V, dtype=np.float32)            # iota
    vpack[0:F_TOT, 1] = np.concatenate([b1, b3, b5])        # conv bias
    vpack[70:120, 2] = -1e30                                # maska: l in {28,29}
    vpack[30:120, 3] = -1e30                                # maskb: l in {30,31}
    vpack[:, 4:] = lin_b.reshape(1, EMB)                    # output bias

    flat = np.ascontiguousarray(word_idxs, dtype=np.int32).reshape(-1)
    common = {
        "char_table": char_table,
        "wpack": wpack.astype(mmnp),
        "vpack": vpack,
    }
    in_maps = []
    for i in range(N_CORES):
        shard = flat[i * W : (i + 1) * W].reshape(128, WPC)
        in_maps.append({"widx": np.ascontiguousarray(shard), **common})
    return in_maps


def run(inputs, trace=False, **kw):
    nc = _build_program()
    in_maps = _host_prep(inputs)
    res = run_bass_kernel_spmd(
        nc, in_maps, core_ids=list(range(N_CORES)), trace=trace, **kw
    )
    out = np.concatenate([r["out"] for r in res.results], axis=0)
    return out.reshape(B, S, EMB).astype(np.float32), res


def kernel(**inputs):
    out, _ = run(inputs, trace=False)
    return out

